# revision 1
# baseline (speedup 1.0000x reference)
"""DWHT (buggy in-place Walsh-Hadamard channel transform + channel shuffle) on 8 trn2 cores.

The whole nn.Module is a fixed linear map on the channel axis:
    y[b, :, h, w] = T @ x[b, :, h, w]
with T a (512, 256) matrix of small integers (|T| <= 13, exactly representable
in bf16).  Batch 64 is sharded 8-ways (data parallel, 8 samples/core); each
core runs a tiled PE matmul: for every sample, y_s (512,784) = T @ x_s (256,784).

Precision strategy (default variant "fp16_lo"): split x = hi + lo where hi is
the bf16 truncation of the fp32 payload (read for FREE via a strided bf16 view
of the upper 2 bytes of each little-endian fp32 word) and lo = fp16(x - hi)
(one DVE op per k-chunk).  T is exact in both bf16 and fp16, so the products
T@hi (bf16 matmul) and T@lo (fp16 matmul) are exact and only fp32 PSUM
accumulation rounds: measured rel err ~6e-7 vs the fp32 reference while the PE
streams at full 16-bit rate.  The kernel is memory-bound: ~19.5 MB HBM traffic
per core ≈ 54 us at ~358 GB/s; the CoreSim cost model places this schedule at
~49 us/core with DMA saturated throughout (model slightly optimistic — it
splits transfer time across the two HWDGE queues).
"""

import os
import sys

import numpy as np

for _p in ("/opt/trn_rl_repo", "/root/.axon_site/_ro/trn_rl_repo"):
    if os.path.isdir(_p) and _p not in sys.path:
        sys.path.append(_p)

B, C_IN, C_OUT, HH, WW = 64, 256, 512, 28, 28
S = HH * WW  # 784
N_CORES = 8
BS = B // N_CORES  # 8 samples per core
N_PASSES, GROUPS = 8, 8

VARIANT = os.environ.get("DWHT_VARIANT", "fp16_lo")

# spatial split per PSUM bank (each chunk <= 512 fp32 = one bank)
N_CHUNKS = ((0, 392), (392, 392))


def _dwht_T() -> np.ndarray:
    """Build the (512, 256) transform matrix by running the reference
    butterfly (including its partial-update in-place semantics) on identity."""
    x = np.zeros((C_OUT, C_IN), np.float64)
    x[:C_IN] = np.eye(C_IN)
    half = C_OUT // 2
    for _ in range(N_PASSES):
        top = x[::2] + x[1::2]
        x = x.copy()
        x[:half] = top
        bottom = x[::2] - x[1::2]
        x[half:] = bottom
    # channel shuffle with groups=8
    x = x.reshape(GROUPS, C_OUT // GROUPS, C_IN).transpose(1, 0, 2).reshape(C_OUT, C_IN)
    return x


def _build(variant, reps=1):
    import concourse.mybir as mybir
    from concourse import bacc
    from concourse.tile import TileContext

    f32 = mybir.dt.float32
    bf16 = mybir.dt.bfloat16
    fp16 = mybir.dt.float16
    f32r = mybir.dt.float32r

    tt_dt = bf16 if variant in ("bf16_split", "fp16_lo") else f32

    nc = bacc.Bacc(None, target_bir_lowering=False)
    x = nc.dram_tensor("x", (BS, C_IN, S), f32, kind="ExternalInput")
    tt = nc.dram_tensor("tt", (C_IN, C_OUT), tt_dt, kind="ExternalInput")
    y = nc.dram_tensor("y", (BS, C_OUT, S), f32, kind="ExternalOutput")

    with TileContext(nc) as tc:
        with (
            tc.tile_pool(name="w", bufs=1) as wp,
            tc.tile_pool(name="io", bufs=3) as io,
            tc.tile_pool(name="ps", bufs=8, space="PSUM") as pp,
        ):
            tts = []
            ttr = []
            tth = []
            for k in range(2):
                t = wp.tile([128, C_OUT], tt_dt, tag=f"tt{k}")
                nc.sync.dma_start(out=t[:], in_=tt[k * 128 : (k + 1) * 128, :])
                tts.append(t)
                if variant == "f32r":
                    r = wp.tile([128, C_OUT], f32r, tag=f"ttr{k}")
                    nc.vector.tensor_copy(r[:], t[:])
                    ttr.append(r)
                if variant == "fp16_lo":
                    # T is small integers: exact in fp16 as well
                    h = wp.tile([128, C_OUT], fp16, tag=f"tth{k}")
                    nc.vector.tensor_copy(h[:], t[:])
                    tth.append(h)

            if variant in ("bf16_split", "fp16_lo"):
                nwarm = int(os.environ.get("DWHT_WARM", "0"))
                if nwarm:
                    warm = pp.tile([128, 392], f32, tag="ps", name="warm")
                    for wi in range(nwarm):
                        nc.tensor.matmul(
                            warm[:],
                            tts[0][:, 0:128],
                            tts[0][:, 0:392],
                            start=(wi == 0),
                            stop=(wi == nwarm - 1),
                        )

            in_combine = os.environ.get("DWHT_IN_COMBINE", "0") == "1"
            out_combine = os.environ.get("DWHT_OUT_SAMPLE", "0") == "1"
            sample_seq = [s for _ in range(reps) for s in range(BS)]
            for si, s in enumerate(sample_seq):
                last_sample = si == len(sample_seq) - 1
                first_sample = si == 0
                xsk = []
                if first_sample and not in_combine:
                    for k in range(2):
                        xs = io.tile([128, S], f32, tag="xs", bufs=8)
                        for n0, nsz in N_CHUNKS:
                            nc.scalar.dma_start(
                                out=xs[:, n0 : n0 + nsz],
                                in_=x[s, k * 128 : (k + 1) * 128, n0 : n0 + nsz],
                            )
                        xsk.append(xs)
                elif in_combine:
                    # one 802KB DMA per sample: [128, 2, 784] with k in free dim
                    xs2 = io.tile([128, 2, S], f32, tag="xs", bufs=8, name="xs2")
                    nc.scalar.dma_start(
                        out=xs2[:], in_=x[s].rearrange("(a p) f -> p a f", p=128)
                    )
                    xsk = [xs2[:, 0], xs2[:, 1]]
                else:
                    for k in range(2):
                        xs = io.tile([128, S], f32, tag="xs", bufs=8)
                        nc.scalar.dma_start(
                            out=xs[:], in_=x[s, k * 128 : (k + 1) * 128, :]
                        )
                        xsk.append(xs)

                # passes: list of (rhs_ap, weights_tile) accumulated into PSUM
                if variant in ("bf16_split", "fp16_lo"):
                    lo_dt = bf16 if variant == "bf16_split" else fp16
                    lo_tt = tts if variant == "bf16_split" else tth
                    # hi: upper 2 bytes of each little-endian fp32 word
                    xhk = [
                        xs.bitcast(bf16).rearrange("p (f two) -> p f two", two=2)[
                            :, :, 1
                        ]
                        for xs in xsk
                    ]
                    xlk = []
                    for k in range(2):
                        xl = io.tile([128, S], lo_dt, tag="xl", bufs=8)
                        if first_sample:
                            for n0, nsz in N_CHUNKS:
                                nsl = slice(n0, n0 + nsz)
                                nc.vector.tensor_sub(
                                    xl[:, nsl], xsk[k][:, nsl], xhk[k][:, nsl]
                                )
                        else:
                            nc.vector.tensor_sub(xl[:], xsk[k][:], xhk[k])
                        xlk.append(xl)
                    # xl first: the group-opening matmul's psum-slot-release
                    # wait and its rhs-ready wait are then the same DVE sem
                    # (the MM ISA slot can encode only one sync wait).
                    passes = [
                        (xlk[0][:], lo_tt[0]),
                        (xhk[0], tts[0]),
                        (xlk[1][:], lo_tt[1]),
                        (xhk[1], tts[1]),
                    ]
                elif variant == "f32r":
                    xrk = []
                    for k in range(2):
                        xr = io.tile([128, S], f32r, tag="xr", bufs=8)
                        nc.vector.tensor_copy(xr[:], xsk[k][:])
                        xrk.append(xr)
                    passes = [(xrk[0][:], ttr[0]), (xrk[1][:], ttr[1])]
                elif variant == "f32":
                    passes = [(xsk[0][:], tts[0]), (xsk[1][:], tts[1])]
                else:
                    raise ValueError(variant)

                ys4 = None
                if out_combine:
                    ys4 = io.tile([128, 4, S], f32, tag="ys4", bufs=3, name="ys4")
                for m in range(C_OUT // 128):
                    msl = slice(m * 128, (m + 1) * 128)
                    ysm = None
                    if not out_combine:
                        ysm = io.tile([128, S], f32, tag="ysm", bufs=6, name="ysm")
                    for ni, (n0, nsz) in enumerate(N_CHUNKS):
                        nsl = slice(n0, n0 + nsz)
                        ps = pp.tile([128, nsz], f32, tag="ps")
                        for i, (src, w) in enumerate(passes):
                            nc.tensor.matmul(
                                ps[:],
                                w[:, msl],
                                src[:, nsl],
                                start=(i == 0),
                                stop=(i == len(passes) - 1),
                            )
                        dst = ys4[:, m, nsl] if out_combine else ysm[:, nsl]
                        # balance PSUM->SBUF copies across DVE and ACT; keep the
                        # kernel's final copies on DVE (faster) to shorten the tail
                        if last_sample:
                            # run the two chunks' copy->DMA chains on disjoint
                            # engine/queue pairs so the kernel tail is one
                            # chunk long, not two
                            if ni == 0:
                                nc.vector.tensor_copy(dst, ps[:])
                                nc.sync.dma_start(out=y[s, msl, nsl], in_=dst)
                            else:
                                nc.scalar.copy(dst, ps[:])
                                nc.scalar.dma_start(out=y[s, msl, nsl], in_=dst)
                        elif (m * len(N_CHUNKS) + ni) % 2 == 0:
                            nc.vector.tensor_copy(dst, ps[:])
                        else:
                            nc.scalar.copy(dst, ps[:])
                    if not out_combine and not last_sample:
                        nc.sync.dma_start(out=y[s, msl, :], in_=ysm[:])
                if out_combine:
                    nc.sync.dma_start(
                        out=y[s].rearrange("(m p) f -> p m f", p=128), in_=ys4[:]
                    )

    nc.compile()
    return nc


_cache = {}


def _get_nc(variant, reps=1):
    key = (variant, reps)
    if key not in _cache:
        _cache[key] = _build(variant, reps)
    return _cache[key]


def _in_maps(x_np, variant):
    import ml_dtypes

    T = _dwht_T()
    ttT = np.ascontiguousarray(T.T)  # (256, 512), lhsT layout
    if variant in ("bf16_split", "fp16_lo"):
        tt_np = ttT.astype(ml_dtypes.bfloat16)
    else:
        tt_np = ttT.astype(np.float32)
    return [
        {"x": x_np[i * BS : (i + 1) * BS], "tt": tt_np} for i in range(N_CORES)
    ]


def _run(x_np, variant=None, trace=False, reps=1):
    from concourse.bass_utils import run_bass_kernel_spmd

    variant = variant or VARIANT
    nc = _get_nc(variant, reps)
    res = run_bass_kernel_spmd(
        nc, _in_maps(x_np, variant), list(range(N_CORES)), trace=trace
    )
    y = np.stack([r["y"] for r in res.results]).reshape(B, C_OUT, HH, WW)
    return y, res


def kernel(x: np.ndarray) -> np.ndarray:
    x_np = np.ascontiguousarray(np.asarray(x), dtype=np.float32).reshape(B, C_IN, S)
    y, _ = _run(x_np)
    return y

